# revision 1
# baseline (speedup 1.0000x reference)
"""Trainium2 Bass kernel for nn_Att_76381698392129.

kernel(**inputs) -> np.ndarray, self-contained.

Reference math:
    v     = x @ value_w.T                      [B, N, 3]
    score = (key_w @ query_w) / 16             [N, N]
    l1    = sum_o |score[i, o]|
    s_n   = score / max(l1, 1e-12)
    y     = einsum("io,bid->bod", s_n, v)      [B, N, 3]

Factored algorithm (never materializes the N x N score matrix):
    raw_l1[i] = sum_o |(key_w @ query_w)[i, o]|         (the only big matmul)
    r[i]      = 1 / max(raw_l1[i], 16e-12)              (the /16 scale cancels)
    T         = key_w.T @ (X * r)       [H, B*3],  X[i, (b,d)] = x[b, i, d]
    Tv[h,(b,e)] = sum_d T[h,(b,d)] vw[e,d]              (3x3 value map)
    y[b,o,e]  = (query_w.T @ Tv)[o, (b,e)]

Distribution (8 NeuronCores):
  Phase A - rows (i) sharded 8 ways: each core computes raw_l1 for its rows
  and the partial Tv_c = key_w[shard].T @ (X[shard] * r).  The host sums the
  8 partial [256, 192] Tv_c (the gather step of this contraction sharding).
  Phase B - output rows (o) sharded 8 ways: Y[o-shard] = qw[:, o-shard].T @ Tv.
  Each phase runs as 8 independent single-device executions (no collectives).
"""

import os
from contextlib import ExitStack

import numpy as np

import concourse.bass as bass
import concourse.mybir as mybir
import concourse.tile as tile

F32 = mybir.dt.float32
F32R = mybir.dt.float32r
AX = mybir.AxisListType
ALU = mybir.AluOpType
ACTF = mybir.ActivationFunctionType

N = 5023
H_DIM = 256
B = 64
BD = B * 3
N_CORES = 8
N_PAD = 5120
S = N_PAD // N_CORES

LAST_HW_EXEC_NS = None
LAST_PHASE_A_NS = None
LAST_PHASE_B_NS = None

_PATCHED = False


def _patch_tile_drain():
    """This walrus build rejects >1 sync-wait on an InstDrain; re-emit the
    final drain's waits as individual wait_ge instructions."""
    global _PATCHED
    if _PATCHED:
        return
    _PATCHED = True
    import bass_rust

    def _drain_and_barrier(self, tick_clock, wait_clock):
        nc = self.nc
        probe = nc.sync.nop(nofuse=True, hint="drain_waits")
        wait_clock.add_sem_waits(
            probe.ins, bass_rust.ScopedClock({None: tick_clock.global_clock})
        )
        waits = list(probe.ins.sync_info.on_wait or []) if probe.ins.sync_info else []
        if probe.ins.sync_info is not None:
            probe.ins.sync_info.on_wait = []
        handles = {h.num: h for h in self.sems.allocated().values()}
        for w in waits:
            h = handles.get(w.id)
            assert h is not None, f"no handle for sem wait {w}"
            assert w.wait_mode == "sem-ge-imm", w
            nc.sync.wait_ge(h, w.wait_value)
        nc.sync.drain()
        nc.all_engine_barrier()
        popped = nc._tile_sem_poison_stack.pop()
        assert popped is self._sem_poison
        nc.clear_and_free_semaphores(list(self.sems.allocated().values()))
        nc.all_engine_barrier()

    tile.TileContext._drain_and_barrier = _drain_and_barrier




def _fix_multiwait(nc, max_waits=1):
    """This walrus build accepts at most one sync-wait command per
    instruction; peel extra waits onto same-engine nops just ahead."""
    f = nc.m.functions[0]
    all_blocks = list(f.blocks)
    for blk in all_blocks:
        insts = blk.instructions
        new = []
        for inst in insts:
            si = inst.sync_info
            w = list(si.on_wait) if si and si.on_wait else []
            if len(w) > max_waits:
                keep = w[-max_waits:]
                for extra in w[:-max_waits]:
                    nop = nc.engines[inst.engine].nop(
                        nofuse=True, hint="waitfix").ins
                    removed = False
                    for b2 in all_blocks:
                        l2 = b2.instructions
                        for k in range(len(l2) - 1, -1, -1):
                            if l2[k] is nop:
                                del l2[k]
                                removed = True
                                break
                        if removed:
                            break
                    assert removed, "waitfix nop not found in any block"
                    if nop.sync_info is None:
                        nop.sync_info = mybir.SyncInfo(on_wait=[extra],
                                                       on_update=[])
                    else:
                        nop.sync_info.on_wait = [extra]
                    new.append(nop)
                si.on_wait = keep
            new.append(inst)
        insts[:] = new
    return nc


def _build_phase_a(score_dt="bf16", act_frac=0.5):
    MT = S // 128
    OT = N_PAD // 512
    HT = H_DIM // 128
    QCH = 1024
    NCH = N_PAD // QCH

    nc = bass.Bass("TRN2", target_bir_lowering=False, debug=False)
    sdt = {"bf16": mybir.dt.bfloat16, "f32r": F32R, "f32": F32}[score_dt]
    qw_d = nc.dram_tensor("qw", [H_DIM, N_PAD], sdt, kind="ExternalInput")
    kwt_d = nc.dram_tensor("kwt", [H_DIM, S], sdt, kind="ExternalInput")
    kw_d = nc.dram_tensor("kw", [S, H_DIM], F32, kind="ExternalInput")
    xs_d = nc.dram_tensor("xs", [S, BD], F32, kind="ExternalInput")
    vw_d = nc.dram_tensor("vw", [3, 3], F32, kind="ExternalInput")
    tv_d = nc.dram_tensor("tv", [H_DIM, BD], F32, kind="ExternalOutput")

    n_act = round(OT * act_frac)

    with tile.TileContext(nc) as tc, ExitStack() as ctx:
        sb = ctx.enter_context(tc.tile_pool(name="sb", bufs=1))
        scr_pool = ctx.enter_context(tc.tile_pool(name="scr", bufs=2))
        ps_pool = ctx.enter_context(tc.tile_pool(name="ps", bufs=6, space="PSUM"))
        t_pool = ctx.enter_context(tc.tile_pool(name="tps", bufs=1, space="PSUM"))

        # kwt as one [128, HT*S] tile: dram row-block h -> sbuf cols [h*S, h*S+S)
        kwt_all = sb.tile([128, HT * S], sdt, name="kwt_all", tag="kwt_all")
        nc.sync.dma_start(kwt_all[:].rearrange("p (h s) -> p h s", h=HT),
                          kwt_d.ap().rearrange("(h p) s -> p h s", h=HT))
        kwt_sb = [kwt_all[:, h * S:(h + 1) * S] for h in range(HT)]

        # qw chunk DMAs, interleaved h0/h1 so early o-tiles unblock first
        qw_sb = [[None] * NCH for _ in range(HT)]
        qw_order = []
        for c in range(NCH):
            for h in range(HT):
                t = sb.tile([128, QCH], sdt, name=f"qw{h}_{c}", tag=f"qw{h}_{c}")
                qw_sb[h][c] = t
                qw_order.append((h, c))
        for h, c in qw_order[:2]:
            nc.sync.dma_start(
                qw_sb[h][c][:],
                qw_d.ap()[h * 128:(h + 1) * 128, c * QCH:(c + 1) * QCH])

        xs_all = sb.tile([128, MT * BD], F32, name="xs_all", tag="xs_all")
        nc.sync.dma_start(xs_all[:].rearrange("p (m d) -> p m d", m=MT),
                          xs_d.ap().rearrange("(m p) d -> p m d", m=MT))
        xs_sb = [xs_all[:, m * BD:(m + 1) * BD] for m in range(MT)]

        kw_all = sb.tile([128, MT * H_DIM], F32, name="kw_all", tag="kw_all")
        nc.sync.dma_start(kw_all[:].rearrange("p (m h) -> p m h", m=MT),
                          kw_d.ap().rearrange("(m p) h -> p m h", m=MT))
        kw_sb = [kw_all[:, m * H_DIM:(m + 1) * H_DIM] for m in range(MT)]

        for h, c in qw_order[2:]:
            nc.sync.dma_start(
                qw_sb[h][c][:],
                qw_d.ap()[h * 128:(h + 1) * 128, c * QCH:(c + 1) * QCH])

        # vw_row DMA issued early (tiny); the broadcast matmul itself is
        # emitted after the m-loop so it does not head-block PE's queue
        vw_row = sb.tile([1, 9], F32, name="vw_row", tag="vw_row")
        nc.sync.dma_start(vw_row[:], vw_d.ap().rearrange("(o a) b -> o (a b)", o=1))

        t_ps = [t_pool.tile([128, BD], F32, name=f"tps{h}", tag=f"tps{h}")
                for h in range(HT)]

        for m in range(MT):
            part = scr_pool.tile([128, 16], F32, name="part", tag="part")
            for o in range(OT):
                ps = ps_pool.tile([128, 512], F32, name="ps", tag="ps")
                for h in range(HT):
                    nc.tensor.matmul(
                        ps[:],
                        kwt_sb[h][:, m * 128:(m + 1) * 128],
                        qw_sb[h][o // 2][:, (o % 2) * 512:(o % 2) * 512 + 512],
                        start=(h == 0),
                        stop=(h == HT - 1),
                    )
                if o < n_act:
                    scr = scr_pool.tile([128, 512], F32, name="scr", tag="scr")
                    nc.scalar.activation(
                        scr[:], ps[:], ACTF.Abs, accum_out=part[:, o:o + 1])
                else:
                    nc.vector.tensor_reduce(
                        part[:, o:o + 1], ps[:], axis=AX.X, op=ALU.add,
                        apply_absolute_value=True)

            l1 = scr_pool.tile([128, 1], F32, name="l1", tag="l1")
            nc.vector.tensor_reduce(l1[:], part[:, 0:OT], axis=AX.X, op=ALU.add)
            nc.vector.tensor_scalar_max(l1[:], l1[:], 1.6e-11)
            r = scr_pool.tile([128, 1], F32, name="r", tag="r")
            nc.vector.reciprocal(r[:], l1[:])

            xsc = scr_pool.tile([128, BD], F32, name="xsc", tag="xsc")
            nc.vector.tensor_scalar_mul(xsc[:], xs_sb[m][:], r[:])

            for h in range(HT):
                nc.tensor.matmul(
                    t_ps[h][:],
                    kw_sb[m][:, h * 128:(h + 1) * 128],
                    xsc[:],
                    start=(m == 0),
                    stop=(m == MT - 1),
                )

        # broadcast value_w across partitions via a K=1 matmul with ones
        ones_row = sb.tile([1, 128], F32, name="ones_row", tag="ones_row")
        nc.vector.memset(ones_row[:], 1.0)
        vw_ps = ps_pool.tile([128, 9], F32, name="vw_ps", tag="ps")
        nc.tensor.matmul(vw_ps[:], ones_row[:], vw_row[:], start=True, stop=True)
        vw_b = sb.tile([128, 9], F32, name="vw_b", tag="vw_b")
        nc.vector.tensor_copy(vw_b[:], vw_ps[:])

        tv_all = sb.tile([128, HT * BD], F32, name="tv_all", tag="tv_all")
        for h in range(HT):
            tp3 = t_ps[h][:].rearrange("p (b d) -> p b d", d=3)
            tv3 = tv_all[:, h * BD:(h + 1) * BD].rearrange("p (b d) -> p b d", d=3)
            for e in range(3):
                a = scr_pool.tile([128, BD // 3], F32, name="vm_a", tag="vm_a")
                bt = scr_pool.tile([128, BD // 3], F32, name="vm_b", tag="vm_b")
                nc.vector.tensor_scalar_mul(a[:], tp3[:, :, 0],
                                            vw_b[:, 3 * e:3 * e + 1])
                nc.vector.scalar_tensor_tensor(
                    bt[:], tp3[:, :, 1], vw_b[:, 3 * e + 1:3 * e + 2], a[:],
                    op0=ALU.mult, op1=ALU.add)
                nc.vector.scalar_tensor_tensor(
                    tv3[:, :, e], tp3[:, :, 2], vw_b[:, 3 * e + 2:3 * e + 3], bt[:],
                    op0=ALU.mult, op1=ALU.add)
        nc.sync.dma_start(tv_d.ap().rearrange("(h p) d -> p h d", h=HT),
                          tv_all[:].rearrange("p (h d) -> p h d", h=HT))

    return _fix_multiwait(nc)


def _build_phase_b():
    MT = S // 128
    HT = H_DIM // 128

    nc = bass.Bass("TRN2", target_bir_lowering=False, debug=False)
    qwy_d = nc.dram_tensor("qwy", [H_DIM, S], F32, kind="ExternalInput")
    ts_d = nc.dram_tensor("tsum", [H_DIM, BD], F32, kind="ExternalInput")
    y_d = nc.dram_tensor("y", [S, BD], F32, kind="ExternalOutput")

    with tile.TileContext(nc) as tc, ExitStack() as ctx:
        sb = ctx.enter_context(tc.tile_pool(name="sb", bufs=1))
        scr_pool = ctx.enter_context(tc.tile_pool(name="scr", bufs=2))
        ps_pool = ctx.enter_context(tc.tile_pool(name="ps", bufs=4, space="PSUM"))

        ts_all = sb.tile([128, HT * BD], F32, name="ts_all", tag="ts_all")
        nc.sync.dma_start(ts_all[:].rearrange("p (h d) -> p h d", h=HT),
                          ts_d.ap().rearrange("(h p) d -> p h d", h=HT))
        ts_sb = [ts_all[:, h * BD:(h + 1) * BD] for h in range(HT)]

        qwy_all = sb.tile([128, HT * S], F32, name="qwy_all", tag="qwy_all")
        nc.sync.dma_start(qwy_all[:].rearrange("p (h s) -> p h s", h=HT),
                          qwy_d.ap().rearrange("(h p) s -> p h s", h=HT))
        qwy_sb = [qwy_all[:, h * S:(h + 1) * S] for h in range(HT)]

        ysb_all = sb.tile([128, MT * BD], F32, name="ysb_all", tag="ysb_all")
        for ot in range(MT):
            yp = ps_pool.tile([128, BD], F32, name="yp", tag="yp")
            for h in range(HT):
                nc.tensor.matmul(
                    yp[:],
                    qwy_sb[h][:, ot * 128:(ot + 1) * 128],
                    ts_sb[h][:],
                    start=(h == 0),
                    stop=(h == HT - 1),
                )
            nc.vector.tensor_copy(ysb_all[:, ot * BD:(ot + 1) * BD], yp[:])
        nc.sync.dma_start(y_d.ap().rearrange("(m p) d -> p m d", m=MT),
                          ysb_all[:].rearrange("p (m d) -> p m d", m=MT))

    return _fix_multiwait(nc)


_NC_A = None
_NC_B = None


def _get_programs():
    global _NC_A, _NC_B
    if _NC_A is None:
        _patch_tile_drain()
        _NC_A = _build_phase_a()
        _NC_B = _build_phase_b()
    return _NC_A, _NC_B


def _run_phase(nc, in_maps, profile):
    """Run one SPMD phase as 8 independent single-device executions."""
    import jax
    from concourse import bass2jax
    from concourse.bass_utils import run_bass_kernel_spmd

    devices = jax.devices()[:len(in_maps)]
    results = []
    max_ns = None
    for d, (dev, in_map) in enumerate(zip(devices, in_maps)):
        with jax.default_device(dev):
            if profile:
                r = run_bass_kernel_spmd(
                    nc, [in_map], core_ids=[0], trace=True, trace_cores=[d])
                results.append(r.results[0])
                if r.exec_time_ns is not None:
                    max_ns = max(max_ns or 0, r.exec_time_ns)
            else:
                results.append(
                    bass2jax.run_bass_via_pjrt(nc, [in_map], n_cores=1)[0])
    return results, max_ns


def kernel(x, key_w, query_w, value_w):
    global LAST_HW_EXEC_NS, LAST_PHASE_A_NS, LAST_PHASE_B_NS
    x = np.asarray(x, dtype=np.float32)
    key_w = np.asarray(key_w, dtype=np.float32)
    query_w = np.asarray(query_w, dtype=np.float32)
    value_w = np.asarray(value_w, dtype=np.float32)

    profile = os.environ.get("ATT_PROFILE", "0") == "1"
    nc_a, nc_b = _get_programs()

    # ---- host-side sharding (layout prep only) ----
    kw_pad = np.zeros((N_PAD, H_DIM), np.float32)
    kw_pad[:N] = key_w
    qw_pad = np.zeros((H_DIM, N_PAD), np.float32)
    qw_pad[:, :N] = query_w
    kwt_pad = np.ascontiguousarray(kw_pad.T)
    import ml_dtypes
    qw_bf = qw_pad.astype(ml_dtypes.bfloat16)
    kwt_bf = kwt_pad.astype(ml_dtypes.bfloat16)
    x_pad = np.zeros((N_PAD, BD), np.float32)
    x_pad[:N] = np.ascontiguousarray(x.transpose(1, 0, 2)).reshape(N, BD)

    in_maps_a = []
    for c in range(N_CORES):
        sl = slice(c * S, (c + 1) * S)
        in_maps_a.append({
            "qw": qw_bf,
            "kwt": np.ascontiguousarray(kwt_bf[:, sl]),
            "kw": np.ascontiguousarray(kw_pad[sl]),
            "xs": np.ascontiguousarray(x_pad[sl]),
            "vw": value_w,
        })

    res_a, a_ns = _run_phase(nc_a, in_maps_a, profile)
    # gather: sum the 8 partial Tv contributions [256, 192]
    tsum = np.sum([r["tv"] for r in res_a], axis=0).astype(np.float32)

    in_maps_b = [{
        "qwy": np.ascontiguousarray(qw_pad[:, c * S:(c + 1) * S]),
        "tsum": tsum,
    } for c in range(N_CORES)]

    res_b, b_ns = _run_phase(nc_b, in_maps_b, profile)

    y_full = np.concatenate([r["y"] for r in res_b], axis=0)
    y = np.ascontiguousarray(
        y_full[:N].reshape(N, B, 3).transpose(1, 0, 2)).astype(np.float32)

    LAST_PHASE_A_NS = a_ns
    LAST_PHASE_B_NS = b_ns
    LAST_HW_EXEC_NS = (a_ns or 0) + (b_ns or 0) if profile else None
    return y

